# revision 16
# baseline (speedup 1.0000x reference)
"""GQA kernel for Trainium2, sharded over 8 NeuronCores.

Problem: x[2,2048,2048] -> GQA(HQ=16 q-heads, HKV=4 kv-heads, D=128) -> out[2,2048,2048]
Sharding: core c = b*4 + h handles batch b and kv-head group h (4 q-heads).
Wq/Wk/Wv column-sharded per head group, Wo row-sharded; partial outputs
summed on host per batch.

Per-core kernel (bf16 matmul operands, fp32 PSUM accumulation), fully
software-pipelined so the Tensor engine stays dense:

  warmup:  dummy PE transposes during the initial DMA wait (HAM un-throttle)
           + a dummy Exp to pull the ACT table load off the critical path.
  phase 1: qT[512,2048], kT[128,2048], vT[128,2048] projections from xT
           (x DMA'd in 512-column chunks so matmuls start after 2MB).
  phase 2: attention blocks (ib outer, g inner). Per block, a j-level
           interleave: scores(block B, j) + exp(j) + AV/normalize slices of
           block B-1, so the in-order PE always has runnable work while the
           ACT engine (exp, ~1.1us/tile) is the block-cadence limiter.
  phase 3: partial outT[e,i] = WoT @ attnT per 1024-column half, emitted as
           filler into the later attention blocks; bf16 output DMA.
"""

import math

import numpy as np

B = 2
N = 2048
E = 2048
HQ = 16
G = 4
HKV = 4
D = 128
FQ = G * D  # 512 q-features per group
P = 128
NB = N // 512  # 4 moving-dim chunks
ET = E // P  # 16 contraction tiles
JT = N // P  # 16 key tiles
IB2 = N // 1024  # 2 query blocks of 1024
SCALE = 1.0 / math.sqrt(D)

_CACHE: dict = {}


def _build_program():
    import concourse.bacc as bacc
    import concourse.tile as tile
    from concourse import mybir
    from concourse.masks import make_identity

    f32 = mybir.dt.float32
    bf16 = mybir.dt.bfloat16
    nc = bacc.Bacc("TRN2", target_bir_lowering=False)

    # inputs pre-tiled on host for dense DMA
    xC_d = nc.dram_tensor("xC", [NB, P, ET, 512], bf16, kind="ExternalInput")
    wqT_d = nc.dram_tensor("wqT", [P, ET, FQ], bf16, kind="ExternalInput")
    wkT_d = nc.dram_tensor("wkT", [P, ET, D], bf16, kind="ExternalInput")
    wvT_d = nc.dram_tensor("wvT", [P, ET, D], bf16, kind="ExternalInput")
    woT_d = nc.dram_tensor("woT", [P, G, N], bf16, kind="ExternalInput")
    outT_d = nc.dram_tensor("outT", [ET, P, N], bf16, kind="ExternalOutput")

    with tile.TileContext(nc) as tc:
        with tc.tile_pool(name="persist", bufs=1) as persist, \
             tc.tile_pool(name="w1", bufs=1) as w1, \
             tc.tile_pool(name="xcp", bufs=6) as xcp:
            ident = persist.tile([P, P], bf16, tag="ident")
            make_identity(nc, ident)

            qT = [persist.tile([P, N], bf16, name=f"qT{f}", tag=f"qT{f}")
                  for f in range(G)]
            kT = persist.tile([P, N], bf16, tag="kT")
            vTs = persist.tile([P, N], bf16, tag="vTs")
            # v tiles [j][128, 128] + ones column at 128
            va = persist.tile([P, JT, 132], bf16, tag="va")
            attnT = [persist.tile([P, N], bf16, name=f"attnT{g}", tag=f"attnT{g}")
                     for g in range(G)]
            wo_sb = persist.tile([P, G, N], bf16, tag="wo_sb")
            dume = persist.tile([P, 1], f32, tag="dume")

            # ---- warmup: keep PE busy during the input DMA wait so the
            # HAM clock gate opens before phase 1; prime the ACT exp table.
            with tc.tile_pool(name="warm", bufs=1, space="PSUM") as warm:
                wt = warm.tile([P, P], bf16, tag="wt")
                nc.scalar.activation(
                    dume[:], ident[:, 0:1], mybir.ActivationFunctionType.Exp
                )
                for _ in range(14):
                    nc.tensor.transpose(wt[:], ident[:], ident[:])

            # ---------------- phase 1: K/V projections + q-head g0 ----------
            # Q for g1..g3 is deferred into the early attention blocks as PE
            # filler (x chunks re-DMA'd there), so attention/exp starts ~40us
            # earlier and the ACT-gated gaps of the first blocks have work.
            def xc_dma(ch):
                # half-chunk ch = nb*2 + h covering e-tiles h*8..h*8+7 of
                # column block nb; 1MB per DMA, 6-deep ring for pipelining
                nb, h = ch // 2, ch % 2
                xc = xcp.tile([P, 8, 512], bf16, tag="xc")
                e0 = h * 8
                nc.sync.dma_start(out=xc[:, 0:4, :], in_=xC_d[nb, :, e0:e0 + 4, :])
                nc.sync.dma_start(out=xc[:, 4:8, :],
                                  in_=xC_d[nb, :, e0 + 4:e0 + 8, :])
                return xc

            with tc.tile_pool(name="pp", bufs=1, space="PSUM") as pp, \
                 tc.tile_pool(name="pkv", bufs=1, space="PSUM") as pkv, \
                 tc.tile_pool(name="ptr0", bufs=1, space="PSUM") as ptr0:
                wq_sb = w1.tile([P, ET, FQ], bf16, tag="wq_sb")
                wk_sb = w1.tile([P, ET, D], bf16, tag="wk_sb")
                wv_sb = w1.tile([P, ET, D], bf16, tag="wv_sb")
                # first e-slice of the weights, then x chunk 0, then the rest:
                # the first matmuls only wait on ~2.2MB of DMA.
                for e in range(2):
                    nc.sync.dma_start(out=wk_sb[:, e, :], in_=wkT_d[:, e, :])
                    nc.sync.dma_start(out=wv_sb[:, e, :], in_=wvT_d[:, e, :])
                    nc.sync.dma_start(out=wq_sb[:, e, 0:P], in_=wqT_d[:, e, 0:P])
                xch = [xc_dma(0), xc_dma(1)]
                for e in range(2, ET):
                    nc.sync.dma_start(out=wk_sb[:, e, :], in_=wkT_d[:, e, :])
                    nc.sync.dma_start(out=wv_sb[:, e, :], in_=wvT_d[:, e, :])
                    nc.sync.dma_start(out=wq_sb[:, e, 0:P], in_=wqT_d[:, e, 0:P])
                xch += [xc_dma(2), xc_dma(3)]
                # q weights for the deferred heads g1..3 and wo ride behind
                # the x chunks (not needed until the attention blocks)
                for e in range(ET):
                    nc.sync.dma_start(out=wq_sb[:, e, P:FQ],
                                      in_=wqT_d[:, e, P:FQ])
                nc.sync.dma_start(out=wo_sb[:], in_=woT_d[:])

                for nb in range(NB):
                    sl = slice(nb * 512, (nb + 1) * 512)
                    qps = pp.tile([P, 512], f32, tag="qp0", bufs=2)
                    kvps = pkv.tile([P, 1024], f32, tag="kvp", bufs=2)
                    kps = kvps[:, 0:512]
                    vps = kvps[:, 512:1024]
                    for e in range(ET):
                        if e == 8 and nb < 2:  # prefetch one full chunk ahead
                            xch.append(xc_dma(2 * nb + 4))
                            xch.append(xc_dma(2 * nb + 5))
                        xc = xch[2 * nb + e // 8]
                        xce = xc[:, e % 8, :]
                        st = e == 0
                        sp = e == ET - 1
                        nc.tensor.matmul(
                            kps, wk_sb[:, e, :], xce, start=st, stop=sp
                        )
                        nc.tensor.matmul(
                            vps, wv_sb[:, e, :], xce, start=st, stop=sp
                        )
                        nc.tensor.matmul(
                            qps[:],
                            wq_sb[:, e, 0:P],
                            xce,
                            start=st,
                            stop=sp,
                        )
                    nc.vector.tensor_copy(qT[0][:, sl], qps[:])
                    # k/v copies on ACT (idle in phase 1) so DVE stays free
                    nc.scalar.activation(
                        kT[:, sl], kps, mybir.ActivationFunctionType.Copy
                    )
                    nc.scalar.activation(
                        vTs[:, sl], vps, mybir.ActivationFunctionType.Copy
                    )
                    # v transpose for this chunk's 4 key tiles, via a plain
                    # matmul against identity (background weight-load) rather
                    # than is_transpose (whose LDWEIGHTS can't hide)
                    for j in range(nb * 4, nb * 4 + 4):
                        tp = ptr0.tile([P, P], bf16, tag="tp0", bufs=2)
                        nc.tensor.transpose(
                            tp[:], vTs[:, j * P:(j + 1) * P], ident[:]
                        )
                        nc.vector.tensor_copy(va[:, j, 0:P], tp[:])
                nc.vector.memset(va[:, :, P:P + 1], 1.0)

            # ---------------- phase 2 + 3: attention with interleaved
            # output projection ----------------
            blocks = [(ib, g) for ib in range(IB2) for g in range(G)]

            with tc.tile_pool(name="et", bufs=2) as etp, \
                 tc.tile_pool(name="small", bufs=4) as small, \
                 tc.tile_pool(name="op", bufs=4) as op, \
                 tc.tile_pool(name="ps", bufs=2, space="PSUM") as ps, \
                 tc.tile_pool(name="pav", bufs=2, space="PSUM") as pav, \
                 tc.tile_pool(name="ptr", bufs=1, space="PSUM") as ptr, \
                 tc.tile_pool(name="po", bufs=1, space="PSUM") as po:

                ets_of = {}  # bi -> list of 16 ets tiles

                def scores_unit(bi, j):
                    ib, g = blocks[bi]
                    sps = ps.tile([P, 1024], f32, tag="sps")
                    for half in range(2):
                        nc.tensor.matmul(
                            sps[:, half * 512:(half + 1) * 512],
                            kT[:, j * P:(j + 1) * P],
                            qT[g][:, ib * 1024 + half * 512:
                                   ib * 1024 + (half + 1) * 512],
                            start=True,
                            stop=True,
                        )
                    et = etp.tile([P, 1024], bf16, name=f"et{j}", tag=f"et{j}")
                    nc.scalar.activation(
                        et[:],
                        sps[:],
                        mybir.ActivationFunctionType.Exp,
                        scale=SCALE,
                    )
                    ets_of[bi][j] = et

                avp_of = {}  # (bi, isub) -> psum tile

                def av_unit(bi, u):
                    # u in 0..15: two units per isub; unit 2k = j 0..7 of
                    # isub k, unit 2k+1 = j 8..15 + the normalize chain.
                    ib, g = blocks[bi]
                    isub = u // 2
                    ets = ets_of[bi]
                    if u % 2 == 0:
                        avp = pav.tile([P, 132], f32, tag="avp")
                        avp_of[(bi, isub)] = avp
                        jr = range(0, 8)
                    else:
                        avp = avp_of[(bi, isub)]
                        jr = range(8, 16)
                    for j in jr:
                        nc.tensor.matmul(
                            avp[:, 0:129],
                            ets[j][:, isub * P:(isub + 1) * P],
                            va[:, j, 0:129],
                            start=(j == 0),
                            stop=(j == JT - 1),
                        )
                    if u % 2 == 1:
                        rec = small.tile([P, 1], f32, tag="rec")
                        nc.vector.reciprocal(rec[:], avp[:, 128:129])
                        anorm = small.tile([P, P], bf16, tag="anorm")
                        nc.vector.tensor_scalar_mul(anorm[:], avp[:, 0:P], rec[:])
                        trp = ptr.tile([P, P], bf16, tag="trp")
                        nc.tensor.transpose(trp[:], anorm[:], ident[:])
                        col = (ib * 8 + isub) * P
                        nc.vector.tensor_copy(attnT[g][:, col:col + P], trp[:])

                def ph3_sub(eo, nb, pool, ot_pool, ot_tag, pot_tag, pot_bufs):
                    # one PSUM accumulation group + copy + store
                    pot = pool.tile([P, 512], f32, tag=pot_tag, bufs=pot_bufs)
                    for f in range(G):
                        nc.tensor.matmul(
                            pot[:],
                            wo_sb[:, f, eo * P:(eo + 1) * P],
                            attnT[f][:, nb * 512:(nb + 1) * 512],
                            start=(f == 0),
                            stop=(f == G - 1),
                        )
                    ot = ot_pool.tile([P, 512], bf16, tag=ot_tag)
                    nc.vector.tensor_copy(ot[:], pot[:])
                    nc.sync.dma_start(
                        out=outT_d[eo, :, nb * 512:(nb + 1) * 512],
                        in_=ot[:],
                    )

                # Deferred Q projections for g1..g3, hosted as PE filler in
                # blocks 0..2: block bi computes Q[bi+1] in 4 chunks of 4
                # j-steps (4 e-slices per step), x chunks re-DMA'd through
                # the same ring with 2-chunk prefetch.
                qsp = {}

                # Q[g] halves 0-3 (columns 0:1024) are needed by block g;
                # halves 4-7 by block 4+g.  Spreading per this schedule keeps
                # every pre-ph3 block supplied with PE filler.
                QSCHED = {
                    0: [(1, 0), (1, 1), (1, 2), (1, 3), (2, 0), (2, 1)],
                    1: [(2, 2), (2, 3), (3, 0), (3, 1)],
                    2: [(3, 2), (3, 3), (1, 4), (1, 5)],
                    3: [(1, 6), (1, 7), (2, 4), (2, 5)],
                    4: [(2, 6), (2, 7), (3, 4), (3, 5), (3, 6), (3, 7)],
                }
                XORDER = [h for b in range(5) for (g, h) in QSCHED[b]]
                qstate = {"idx": 0}

                def q_sub(gh, sub, xq):
                    # one quarter of a Q[g] column-chunk: 4 contraction MMs
                    g, h = gh
                    if sub == 0:
                        qsp["cur"] = xq.pop(0)
                        nxt = qstate["idx"] + 4
                        if nxt < len(XORDER):
                            xq.append(xc_dma(XORDER[nxt]))
                        qstate["idx"] += 1
                        if h % 2 == 0:
                            qsp["pot"] = po.tile([P, 512], f32, name="qpot",
                                                 tag="pot", bufs=1)
                    xc, pot = qsp["cur"], qsp["pot"]
                    es = (h % 2) * 8 + sub * 4
                    for e in range(es, es + 4):
                        nc.tensor.matmul(
                            pot[:],
                            wq_sb[:, e, g * P:(g + 1) * P],
                            xc[:, e % 8, :],
                            start=(e == 0),
                            stop=(e == ET - 1),
                        )
                    if es == 12:
                        sc = h // 2
                        nc.vector.tensor_copy(
                            qT[g][:, sc * 512:(sc + 1) * 512], pot[:]
                        )

                # ph3 for half 0 can only be emitted once block 3's AV units
                # are all emitted (in-order PE: a waiting filler would block
                # the stream), i.e. from block 5 on. One PSUM group per
                # filler slot so the single po bank recycles between slots.
                half0 = [(eo, nb) for eo in range(ET) for nb in range(2)]

                xq = [xc_dma(h) for h in XORDER[0:4]]
                for bi in range(len(blocks)):
                    ets_of[bi] = [None] * JT
                    qs = QSCHED.get(bi, [])
                    nsub = 2 * len(qs)
                    qslots = {}
                    for k in range(nsub):
                        qslots.setdefault(k * JT // nsub, []).append(
                            (qs[k // 2], k % 2)
                        )
                    for j in range(JT):
                        scores_unit(bi, j)
                        if bi > 0:
                            av_unit(bi - 1, j)
                        for gh, sub in qslots.get(j, []):
                            q_sub(gh, sub, xq)
                        if bi >= 5 and j % 2 == 1 and half0:
                            eo, nb = half0.pop(0)
                            ph3_sub(eo, nb, po, op, "ot", "pot", 1)
                    if bi >= 2:
                        ets_of.pop(bi - 2)
                # last block's AV + some leftover half-0 ph3
                for u in range(JT):
                    av_unit(len(blocks) - 1, u)
                    if u % 2 == 1 and half0:
                        eo, nb = half0.pop(0)
                        ph3_sub(eo, nb, po, op, "ot", "pot", 1)

                tail = half0 + [(eo, 2 + nbh)
                                for eo in range(ET) for nbh in range(2)]

            # ---- phase 3 tail: remaining half-0 + all half-1 subunits,
            # multi-banked PSUM for dense back-to-back groups
            with tc.tile_pool(name="po2", bufs=3, space="PSUM") as po2, \
                 tc.tile_pool(name="op2", bufs=4) as op2:
                for eo, nb in tail:
                    pot = po2.tile([P, 512], f32, tag="pot2")
                    for f in range(G):
                        nc.tensor.matmul(
                            pot[:],
                            wo_sb[:, f, eo * P:(eo + 1) * P],
                            attnT[f][:, nb * 512:(nb + 1) * 512],
                            start=(f == 0),
                            stop=(f == G - 1),
                        )
                    ot = op2.tile([P, 512], bf16, tag="ot2")
                    nc.vector.tensor_copy(ot[:], pot[:])
                    nc.sync.dma_start(
                        out=outT_d[eo, :, nb * 512:(nb + 1) * 512],
                        in_=ot[:],
                    )
    nc.finalize()
    return nc


def _get_program():
    if "nc" not in _CACHE:
        _CACHE["nc"] = _build_program()
    return _CACHE["nc"]


def _make_in_maps(x, Wq, Wk, Wv, Wo):
    import ml_dtypes

    bf = ml_dtypes.bfloat16

    def wtile(w):  # [rows, E] -> [P, ET_rows, rows_per] tiled on partition
        r = w.shape[0]
        return np.ascontiguousarray(
            w.T.reshape(ET, P, r).transpose(1, 0, 2)
        ).astype(bf)

    # x chunks: [NB, P, ET, 512]; element (nb,p,e,c) = x^T[e*128+p, nb*512+c]
    xC = [
        np.ascontiguousarray(
            x[b].T.reshape(ET, P, NB, 512).transpose(2, 1, 0, 3)
        ).astype(bf)
        for b in range(B)
    ]
    in_maps = []
    for c in range(8):
        b, h = c // HKV, c % HKV
        wo = Wo[:, h * FQ:(h + 1) * FQ].T  # [FQ, E]
        in_maps.append({
            "xC": xC[b],
            "wqT": wtile(Wq[h * FQ:(h + 1) * FQ, :]),
            "wkT": wtile(Wk[h * D:(h + 1) * D, :]),
            "wvT": wtile(Wv[h * D:(h + 1) * D, :]),
            "woT": np.ascontiguousarray(
                wo.reshape(G, P, N).transpose(1, 0, 2)
            ).astype(bf),
        })
    return in_maps


def run_spmd(in_maps, trace=False, **kw):
    from concourse.bass_utils import run_bass_kernel_spmd

    nc = _get_program()
    return run_bass_kernel_spmd(nc, in_maps, list(range(8)), trace=trace, **kw)


def kernel(x, Wq, Wk, Wv, Wo, next_token_only=0, **_ignored):
    x = np.asarray(x, dtype=np.float32)
    Wq = np.asarray(Wq, dtype=np.float32)
    Wk = np.asarray(Wk, dtype=np.float32)
    Wv = np.asarray(Wv, dtype=np.float32)
    Wo = np.asarray(Wo, dtype=np.float32)

    res = run_spmd(_make_in_maps(x, Wq, Wk, Wv, Wo))
    outs = [np.asarray(r["outT"]).reshape(E, N).astype(np.float32)
            for r in res.results]
    full = np.empty((B, N, E), np.float32)
    for b in range(B):
        acc = outs[b * HKV]
        for h in range(1, HKV):
            acc = acc + outs[b * HKV + h]
        full[b] = acc.T
    return full


# revision 18
# speedup vs baseline: 1.0356x; 1.0356x over previous
"""GQA kernel for Trainium2, sharded over 8 NeuronCores.

Problem: x[2,2048,2048] -> GQA(HQ=16 q-heads, HKV=4 kv-heads, D=128) -> out[2,2048,2048]
Sharding: core c = b*4 + h handles batch b and kv-head group h (4 q-heads).
Wq/Wk/Wv column-sharded per head group, Wo row-sharded; partial outputs
summed on host per batch.

Per-core kernel (bf16 matmul operands, fp32 PSUM accumulation), fully
software-pipelined so the Tensor engine stays dense:

  warmup:  dummy PE transposes during the initial DMA wait (HAM un-throttle)
           + a dummy Exp to pull the ACT table load off the critical path.
  phase 1: qT[512,2048], kT[128,2048], vT[128,2048] projections from xT
           (x DMA'd in 512-column chunks so matmuls start after 2MB).
  phase 2: attention blocks (ib outer, g inner). Per block, a j-level
           interleave: scores(block B, j) + exp(j) + AV/normalize slices of
           block B-1, so the in-order PE always has runnable work while the
           ACT engine (exp, ~1.1us/tile) is the block-cadence limiter.
  phase 3: partial outT[e,i] = WoT @ attnT per 1024-column half, emitted as
           filler into the later attention blocks; bf16 output DMA.
"""

import math

import numpy as np

B = 2
N = 2048
E = 2048
HQ = 16
G = 4
HKV = 4
D = 128
FQ = G * D  # 512 q-features per group
P = 128
NB = N // 512  # 4 moving-dim chunks
ET = E // P  # 16 contraction tiles
JT = N // P  # 16 key tiles
IB2 = N // 1024  # 2 query blocks of 1024
SCALE = 1.0 / math.sqrt(D)

_CACHE: dict = {}


def _build_program():
    import concourse.bacc as bacc
    import concourse.tile as tile
    from concourse import mybir
    from concourse.masks import make_identity

    f32 = mybir.dt.float32
    bf16 = mybir.dt.bfloat16
    nc = bacc.Bacc("TRN2", target_bir_lowering=False)

    # inputs pre-tiled on host for dense DMA
    xC_d = nc.dram_tensor("xC", [NB, P, ET, 512], bf16, kind="ExternalInput")
    wqT_d = nc.dram_tensor("wqT", [P, ET, FQ], bf16, kind="ExternalInput")
    wkT_d = nc.dram_tensor("wkT", [P, ET, D], bf16, kind="ExternalInput")
    wvT_d = nc.dram_tensor("wvT", [P, ET, D], bf16, kind="ExternalInput")
    woT_d = nc.dram_tensor("woT", [P, G, N], bf16, kind="ExternalInput")
    outT_d = nc.dram_tensor("outT", [ET, P, N], bf16, kind="ExternalOutput")

    with tile.TileContext(nc) as tc:
        with tc.tile_pool(name="persist", bufs=1) as persist, \
             tc.tile_pool(name="w1", bufs=1) as w1, \
             tc.tile_pool(name="xcp", bufs=6) as xcp:
            ident = persist.tile([P, P], bf16, tag="ident")
            make_identity(nc, ident)

            qT = [persist.tile([P, N], bf16, name=f"qT{f}", tag=f"qT{f}")
                  for f in range(G)]
            kT = persist.tile([P, N], bf16, tag="kT")
            vTs = persist.tile([P, N], bf16, tag="vTs")
            # v tiles [j][128, 128] + ones column at 128
            va = persist.tile([P, JT, 132], bf16, tag="va")
            attnT = [persist.tile([P, N], bf16, name=f"attnT{g}", tag=f"attnT{g}")
                     for g in range(G)]
            wo_sb = persist.tile([P, G, N], bf16, tag="wo_sb")
            dume = persist.tile([P, 1], f32, tag="dume")

            # ---- warmup: keep PE busy during the input DMA wait so the
            # HAM clock gate opens before phase 1; prime the ACT exp table.
            with tc.tile_pool(name="warm", bufs=1, space="PSUM") as warm:
                wt = warm.tile([P, P], bf16, tag="wt")
                nc.scalar.activation(
                    dume[:], ident[:, 0:1], mybir.ActivationFunctionType.Exp
                )
                for _ in range(18):
                    nc.tensor.transpose(wt[:], ident[:], ident[:])

            # ---------------- phase 1: K/V projections + q-head g0 ----------
            # Q for g1..g3 is deferred into the early attention blocks as PE
            # filler (x chunks re-DMA'd there), so attention/exp starts ~40us
            # earlier and the ACT-gated gaps of the first blocks have work.
            def xc_dma(ch):
                # half-chunk ch = nb*2 + h covering e-tiles h*8..h*8+7 of
                # column block nb; 1MB per DMA, 6-deep ring for pipelining
                nb, h = ch // 2, ch % 2
                xc = xcp.tile([P, 8, 512], bf16, tag="xc")
                e0 = h * 8
                nc.sync.dma_start(out=xc[:, 0:4, :], in_=xC_d[nb, :, e0:e0 + 4, :])
                nc.sync.dma_start(out=xc[:, 4:8, :],
                                  in_=xC_d[nb, :, e0 + 4:e0 + 8, :])
                return xc

            with tc.tile_pool(name="pp", bufs=1, space="PSUM") as pp, \
                 tc.tile_pool(name="pkv", bufs=1, space="PSUM") as pkv, \
                 tc.tile_pool(name="ptr0", bufs=1, space="PSUM") as ptr0:
                wq_sb = w1.tile([P, ET, FQ], bf16, tag="wq_sb")
                wk_sb = w1.tile([P, ET, D], bf16, tag="wk_sb")
                wv_sb = w1.tile([P, ET, D], bf16, tag="wv_sb")
                # first e-slice of the weights, then x chunk 0, then the rest:
                # the first matmuls only wait on ~2.2MB of DMA.
                xch = [xc_dma(0)]
                for e in range(2):
                    nc.sync.dma_start(out=wk_sb[:, e, :], in_=wkT_d[:, e, :])
                    nc.sync.dma_start(out=wv_sb[:, e, :], in_=wvT_d[:, e, :])
                    nc.sync.dma_start(out=wq_sb[:, e, 0:P], in_=wqT_d[:, e, 0:P])
                xch.append(xc_dma(1))
                for e in range(2, ET):
                    nc.sync.dma_start(out=wk_sb[:, e, :], in_=wkT_d[:, e, :])
                    nc.sync.dma_start(out=wv_sb[:, e, :], in_=wvT_d[:, e, :])
                    nc.sync.dma_start(out=wq_sb[:, e, 0:P], in_=wqT_d[:, e, 0:P])
                xch += [xc_dma(2), xc_dma(3)]

                for nb in range(NB):
                    sl = slice(nb * 512, (nb + 1) * 512)
                    qps = pp.tile([P, 512], f32, tag="qp0", bufs=2)
                    kvps = pkv.tile([P, 1024], f32, tag="kvp", bufs=2)
                    kps = kvps[:, 0:512]
                    vps = kvps[:, 512:1024]
                    for e in range(ET):
                        if e == 8 and nb < 2:  # prefetch one full chunk ahead
                            xch.append(xc_dma(2 * nb + 4))
                            xch.append(xc_dma(2 * nb + 5))
                        xc = xch[2 * nb + e // 8]
                        xce = xc[:, e % 8, :]
                        st = e == 0
                        sp = e == ET - 1
                        nc.tensor.matmul(
                            kps, wk_sb[:, e, :], xce, start=st, stop=sp
                        )
                        nc.tensor.matmul(
                            vps, wv_sb[:, e, :], xce, start=st, stop=sp
                        )
                        nc.tensor.matmul(
                            qps[:],
                            wq_sb[:, e, 0:P],
                            xce,
                            start=st,
                            stop=sp,
                        )
                    nc.vector.tensor_copy(qT[0][:, sl], qps[:])
                    # k/v copies on ACT (idle in phase 1) so DVE stays free
                    nc.scalar.activation(
                        kT[:, sl], kps, mybir.ActivationFunctionType.Copy
                    )
                    nc.scalar.activation(
                        vTs[:, sl], vps, mybir.ActivationFunctionType.Copy
                    )
                    # v transpose for this chunk's 4 key tiles, via a plain
                    # matmul against identity (background weight-load) rather
                    # than is_transpose (whose LDWEIGHTS can't hide)
                    # v transpose for this chunk's 4 key tiles
                    for j in range(nb * 4, nb * 4 + 4):
                        tp = ptr0.tile([P, P], bf16, tag="tp0", bufs=2)
                        nc.tensor.transpose(
                            tp[:], vTs[:, j * P:(j + 1) * P], ident[:]
                        )
                        nc.vector.tensor_copy(va[:, j, 0:P], tp[:])
                nc.vector.memset(va[:, :, P:P + 1], 1.0)

            # ---------------- phase 2 + 3: attention with interleaved
            # output projection ----------------
            blocks = [(ib, g) for ib in range(IB2) for g in range(G)]

            with tc.tile_pool(name="et", bufs=2) as etp, \
                 tc.tile_pool(name="small", bufs=4) as small, \
                 tc.tile_pool(name="op", bufs=4) as op, \
                 tc.tile_pool(name="ps", bufs=2, space="PSUM") as ps, \
                 tc.tile_pool(name="pav", bufs=2, space="PSUM") as pav, \
                 tc.tile_pool(name="ptr", bufs=1, space="PSUM") as ptr, \
                 tc.tile_pool(name="po", bufs=1, space="PSUM") as po:

                ets_of = {}  # bi -> list of 16 ets tiles

                def scores_unit(bi, j):
                    ib, g = blocks[bi]
                    sps = ps.tile([P, 1024], f32, tag="sps")
                    for half in range(2):
                        nc.tensor.matmul(
                            sps[:, half * 512:(half + 1) * 512],
                            kT[:, j * P:(j + 1) * P],
                            qT[g][:, ib * 1024 + half * 512:
                                   ib * 1024 + (half + 1) * 512],
                            start=True,
                            stop=True,
                        )
                    et = etp.tile([P, 1024], bf16, name=f"et{j}", tag=f"et{j}")
                    nc.scalar.activation(
                        et[:],
                        sps[:],
                        mybir.ActivationFunctionType.Exp,
                        scale=SCALE,
                    )
                    ets_of[bi][j] = et

                avp_of = {}  # (bi, isub) -> psum tile

                def av_unit(bi, u):
                    # u in 0..15: two units per isub; unit 2k = j 0..7 of
                    # isub k, unit 2k+1 = j 8..15 + the normalize chain.
                    ib, g = blocks[bi]
                    isub = u // 2
                    ets = ets_of[bi]
                    if u % 2 == 0:
                        avp = pav.tile([P, 132], f32, tag="avp")
                        avp_of[(bi, isub)] = avp
                        jr = range(0, 8)
                    else:
                        avp = avp_of[(bi, isub)]
                        jr = range(8, 16)
                    for j in jr:
                        nc.tensor.matmul(
                            avp[:, 0:129],
                            ets[j][:, isub * P:(isub + 1) * P],
                            va[:, j, 0:129],
                            start=(j == 0),
                            stop=(j == JT - 1),
                        )
                    if u % 2 == 1:
                        rec = small.tile([P, 1], f32, tag="rec")
                        nc.vector.reciprocal(rec[:], avp[:, 128:129])
                        anorm = small.tile([P, P], bf16, tag="anorm")
                        nc.vector.tensor_scalar_mul(anorm[:], avp[:, 0:P], rec[:])
                        trp = ptr.tile([P, P], bf16, tag="trp")
                        nc.tensor.transpose(trp[:], anorm[:], ident[:])
                        col = (ib * 8 + isub) * P
                        nc.vector.tensor_copy(attnT[g][:, col:col + P], trp[:])

                def ph3_sub(eo, nb, pool, ot_pool, ot_tag, pot_tag, pot_bufs):
                    # one PSUM accumulation group + copy + store
                    pot = pool.tile([P, 512], f32, tag=pot_tag, bufs=pot_bufs)
                    for f in range(G):
                        nc.tensor.matmul(
                            pot[:],
                            wo_sb[:, f, eo * P:(eo + 1) * P],
                            attnT[f][:, nb * 512:(nb + 1) * 512],
                            start=(f == 0),
                            stop=(f == G - 1),
                        )
                    ot = ot_pool.tile([P, 512], bf16, tag=ot_tag)
                    nc.vector.tensor_copy(ot[:], pot[:])
                    nc.sync.dma_start(
                        out=outT_d[eo, :, nb * 512:(nb + 1) * 512],
                        in_=ot[:],
                    )

                # Deferred Q projections for g1..g3, hosted as PE filler in
                # blocks 0..2: block bi computes Q[bi+1] in 4 chunks of 4
                # j-steps (4 e-slices per step), x chunks re-DMA'd through
                # the same ring with 2-chunk prefetch.
                qsp = {}

                # Q[g] halves 0-3 (columns 0:1024) are needed by block g;
                # halves 4-7 by block 4+g.  Spreading per this schedule keeps
                # every pre-ph3 block supplied with PE filler.
                QSCHED = {
                    0: [(1, 0), (1, 1), (1, 2), (1, 3), (2, 0), (2, 1)],
                    1: [(2, 2), (2, 3), (3, 0), (3, 1)],
                    2: [(3, 2), (3, 3), (1, 4), (1, 5)],
                    3: [(1, 6), (1, 7), (2, 4), (2, 5)],
                    4: [(2, 6), (2, 7), (3, 4), (3, 5), (3, 6), (3, 7)],
                }
                XORDER = [h for b in range(5) for (g, h) in QSCHED[b]]
                qstate = {"idx": 0}

                def q_sub(gh, sub, xq):
                    # one quarter of a Q[g] column-chunk: 4 contraction MMs
                    g, h = gh
                    if sub == 0:
                        qsp["cur"] = xq.pop(0)
                        nxt = qstate["idx"] + 4
                        if nxt < len(XORDER):
                            xq.append(xc_dma(XORDER[nxt]))
                        qstate["idx"] += 1
                        if h % 2 == 0:
                            qsp["pot"] = po.tile([P, 512], f32, name="qpot",
                                                 tag="pot", bufs=1)
                    xc, pot = qsp["cur"], qsp["pot"]
                    es = (h % 2) * 8 + sub * 4
                    for e in range(es, es + 4):
                        nc.tensor.matmul(
                            pot[:],
                            wq_sb[:, e, g * P:(g + 1) * P],
                            xc[:, e % 8, :],
                            start=(e == 0),
                            stop=(e == ET - 1),
                        )
                    if es == 12:
                        sc = h // 2
                        nc.vector.tensor_copy(
                            qT[g][:, sc * 512:(sc + 1) * 512], pot[:]
                        )

                # ph3 for half 0 can only be emitted once block 3's AV units
                # are all emitted (in-order PE: a waiting filler would block
                # the stream), i.e. from block 5 on. One PSUM group per
                # filler slot so the single po bank recycles between slots.
                half0 = [(eo, nb) for eo in range(ET) for nb in range(2)]

                xq = [xc_dma(h) for h in XORDER[0:2]]
                # deferred q-head weights for g1/g2 land next, then the rest
                # of the filler x halves, g3 weights, and wo
                for e in range(ET):
                    nc.sync.dma_start(out=wq_sb[:, e, P:3 * P],
                                      in_=wqT_d[:, e, P:3 * P])
                xq += [xc_dma(h) for h in XORDER[2:4]]
                for e in range(ET):
                    nc.sync.dma_start(out=wq_sb[:, e, 3 * P:FQ],
                                      in_=wqT_d[:, e, 3 * P:FQ])
                nc.sync.dma_start(out=wo_sb[:], in_=woT_d[:])
                for bi in range(len(blocks)):
                    ets_of[bi] = [None] * JT
                    qs = QSCHED.get(bi, [])
                    nsub = 2 * len(qs)
                    qslots = {}
                    for k in range(nsub):
                        qslots.setdefault(k * JT // nsub, []).append(
                            (qs[k // 2], k % 2)
                        )
                    for j in range(JT):
                        scores_unit(bi, j)
                        if bi > 0:
                            av_unit(bi - 1, j)
                        for gh, sub in qslots.get(j, []):
                            q_sub(gh, sub, xq)
                        if bi >= 5 and j % 2 == 1 and half0:
                            eo, nb = half0.pop(0)
                            ph3_sub(eo, nb, po, op, "ot", "pot", 1)
                    if bi >= 2:
                        ets_of.pop(bi - 2)
                # last block's AV + some leftover half-0 ph3
                for u in range(JT):
                    av_unit(len(blocks) - 1, u)
                    if u % 2 == 1 and half0:
                        eo, nb = half0.pop(0)
                        ph3_sub(eo, nb, po, op, "ot", "pot", 1)

                tail = half0 + [(eo, 2 + nbh)
                                for eo in range(ET) for nbh in range(2)]

            # ---- phase 3 tail: remaining half-0 + all half-1 subunits,
            # multi-banked PSUM for dense back-to-back groups
            with tc.tile_pool(name="po2", bufs=3, space="PSUM") as po2, \
                 tc.tile_pool(name="op2", bufs=4) as op2:
                for eo, nb in tail:
                    pot = po2.tile([P, 512], f32, tag="pot2")
                    for f in range(G):
                        nc.tensor.matmul(
                            pot[:],
                            wo_sb[:, f, eo * P:(eo + 1) * P],
                            attnT[f][:, nb * 512:(nb + 1) * 512],
                            start=(f == 0),
                            stop=(f == G - 1),
                        )
                    ot = op2.tile([P, 512], bf16, tag="ot2")
                    nc.vector.tensor_copy(ot[:], pot[:])
                    nc.sync.dma_start(
                        out=outT_d[eo, :, nb * 512:(nb + 1) * 512],
                        in_=ot[:],
                    )
    nc.finalize()
    return nc


def _get_program():
    if "nc" not in _CACHE:
        _CACHE["nc"] = _build_program()
    return _CACHE["nc"]


def _make_in_maps(x, Wq, Wk, Wv, Wo):
    import ml_dtypes

    bf = ml_dtypes.bfloat16

    def wtile(w):  # [rows, E] -> [P, ET_rows, rows_per] tiled on partition
        r = w.shape[0]
        return np.ascontiguousarray(
            w.T.reshape(ET, P, r).transpose(1, 0, 2)
        ).astype(bf)

    # x chunks: [NB, P, ET, 512]; element (nb,p,e,c) = x^T[e*128+p, nb*512+c]
    xC = [
        np.ascontiguousarray(
            x[b].T.reshape(ET, P, NB, 512).transpose(2, 1, 0, 3)
        ).astype(bf)
        for b in range(B)
    ]
    in_maps = []
    for c in range(8):
        b, h = c // HKV, c % HKV
        wo = Wo[:, h * FQ:(h + 1) * FQ].T  # [FQ, E]
        in_maps.append({
            "xC": xC[b],
            "wqT": wtile(Wq[h * FQ:(h + 1) * FQ, :]),
            "wkT": wtile(Wk[h * D:(h + 1) * D, :]),
            "wvT": wtile(Wv[h * D:(h + 1) * D, :]),
            "woT": np.ascontiguousarray(
                wo.reshape(G, P, N).transpose(1, 0, 2)
            ).astype(bf),
        })
    return in_maps


def run_spmd(in_maps, trace=False, **kw):
    from concourse.bass_utils import run_bass_kernel_spmd

    nc = _get_program()
    return run_bass_kernel_spmd(nc, in_maps, list(range(8)), trace=trace, **kw)


def kernel(x, Wq, Wk, Wv, Wo, next_token_only=0, **_ignored):
    x = np.asarray(x, dtype=np.float32)
    Wq = np.asarray(Wq, dtype=np.float32)
    Wk = np.asarray(Wk, dtype=np.float32)
    Wv = np.asarray(Wv, dtype=np.float32)
    Wo = np.asarray(Wo, dtype=np.float32)

    res = run_spmd(_make_in_maps(x, Wq, Wk, Wv, Wo))
    outs = [np.asarray(r["outT"]).reshape(E, N).astype(np.float32)
            for r in res.results]
    full = np.empty((B, N, E), np.float32)
    for b in range(B):
        acc = outs[b * HKV]
        for h in range(1, HKV):
            acc = acc + outs[b * HKV + h]
        full[b] = acc.T
    return full


# revision 20
# speedup vs baseline: 1.0997x; 1.0619x over previous
"""GQA kernel for Trainium2, sharded over 8 NeuronCores.

Problem: x[2,2048,2048] -> GQA(HQ=16 q-heads, HKV=4 kv-heads, D=128) -> out[2,2048,2048]
Sharding: core c = b*4 + h handles batch b and kv-head group h (4 q-heads).
Wq/Wk/Wv column-sharded per head group, Wo row-sharded; partial outputs
summed on host per batch.

Per-core kernel (bf16 matmul operands, fp32 PSUM accumulation), fully
software-pipelined so the Tensor engine stays dense:

  warmup:  dummy PE transposes during the initial DMA wait (HAM un-throttle)
           + a dummy Exp to pull the ACT table load off the critical path.
  phase 1: qT[512,2048], kT[128,2048], vT[128,2048] projections from xT
           (x DMA'd in 512-column chunks so matmuls start after 2MB).
  phase 2: attention blocks (ib outer, g inner). Per block, a j-level
           interleave: scores(block B, j) + exp(j) + AV/normalize slices of
           block B-1, so the in-order PE always has runnable work while the
           ACT engine (exp, ~1.1us/tile) is the block-cadence limiter.
  phase 3: partial outT[e,i] = WoT @ attnT per 1024-column half, emitted as
           filler into the later attention blocks; bf16 output DMA.
"""

import math

import numpy as np

B = 2
N = 2048
E = 2048
HQ = 16
G = 4
HKV = 4
D = 128
FQ = G * D  # 512 q-features per group
P = 128
NB = N // 512  # 4 moving-dim chunks
ET = E // P  # 16 contraction tiles
JT = N // P  # 16 key tiles
IB2 = N // 1024  # 2 query blocks of 1024
SCALE = 1.0 / math.sqrt(D)

_CACHE: dict = {}


def _build_program():
    import concourse.bacc as bacc
    import concourse.tile as tile
    from concourse import mybir
    from concourse.masks import make_identity

    f32 = mybir.dt.float32
    bf16 = mybir.dt.bfloat16
    nc = bacc.Bacc("TRN2", target_bir_lowering=False)

    # inputs pre-tiled on host for dense DMA
    xC_d = nc.dram_tensor("xC", [NB, P, ET, 512], bf16, kind="ExternalInput")
    wqT_d = nc.dram_tensor("wqT", [P, ET, FQ], bf16, kind="ExternalInput")
    wkT_d = nc.dram_tensor("wkT", [P, ET, D], bf16, kind="ExternalInput")
    wvT_d = nc.dram_tensor("wvT", [P, ET, D], bf16, kind="ExternalInput")
    woT_d = nc.dram_tensor("woT", [P, G, N], bf16, kind="ExternalInput")
    outT_d = nc.dram_tensor("outT", [ET, P, N], bf16, kind="ExternalOutput")

    with tile.TileContext(nc) as tc:
        with tc.tile_pool(name="persist", bufs=1) as persist, \
             tc.tile_pool(name="w1", bufs=1) as w1, \
             tc.tile_pool(name="xcp", bufs=6) as xcp:
            ident = persist.tile([P, P], bf16, tag="ident")
            make_identity(nc, ident)

            qT = [persist.tile([P, N], bf16, name=f"qT{f}", tag=f"qT{f}")
                  for f in range(G)]
            kT = persist.tile([P, N], bf16, tag="kT")
            vTs = persist.tile([P, N], bf16, tag="vTs")
            # v tiles [j][128, 128] + ones column at 128
            va = persist.tile([P, JT, 132], bf16, tag="va")
            attnT = [persist.tile([P, N], bf16, name=f"attnT{g}", tag=f"attnT{g}")
                     for g in range(G)]
            wo_sb = persist.tile([P, G, N], bf16, tag="wo_sb")
            dume = persist.tile([P, 1], f32, tag="dume")

            # ---- warmup: keep PE busy during the input DMA wait so the
            # HAM clock gate opens before phase 1; prime the ACT exp table.
            with tc.tile_pool(name="warm", bufs=1, space="PSUM") as warm:
                wt = warm.tile([P, P], bf16, tag="wt")
                nc.scalar.activation(
                    dume[:], ident[:, 0:1], mybir.ActivationFunctionType.Exp
                )
                for _ in range(18):
                    nc.tensor.transpose(wt[:], ident[:], ident[:])

            # ---------------- phase 1: K/V projections + q-head g0 ----------
            # Q for g1..g3 is deferred into the early attention blocks as PE
            # filler (x chunks re-DMA'd there), so attention/exp starts ~40us
            # earlier and the ACT-gated gaps of the first blocks have work.
            def xc_dma(ch, split=True):
                # half-chunk ch = nb*2 + h covering e-tiles h*8..h*8+7 of
                # column block nb; 6-deep ring for pipelining.  split=True
                # issues two quarter transfers (lower latency, for phase 1);
                # refetches for the deferred Q are latency-tolerant.
                nb, h = ch // 2, ch % 2
                xc = xcp.tile([P, 8, 512], bf16, tag="xc")
                e0 = h * 8
                if split:
                    nc.sync.dma_start(out=xc[:, 0:4, :],
                                      in_=xC_d[nb, :, e0:e0 + 4, :])
                    nc.sync.dma_start(out=xc[:, 4:8, :],
                                      in_=xC_d[nb, :, e0 + 4:e0 + 8, :])
                else:
                    nc.sync.dma_start(out=xc[:],
                                      in_=xC_d[nb, :, e0:e0 + 8, :])
                return xc

            with tc.tile_pool(name="pp", bufs=1, space="PSUM") as pp, \
                 tc.tile_pool(name="pkv", bufs=1, space="PSUM") as pkv, \
                 tc.tile_pool(name="ptr0", bufs=1, space="PSUM") as ptr0:
                wq_sb = w1.tile([P, ET, FQ], bf16, tag="wq_sb")
                wk_sb = w1.tile([P, ET, D], bf16, tag="wk_sb")
                wv_sb = w1.tile([P, ET, D], bf16, tag="wv_sb")
                # first e-slice of the weights, then x chunk 0, then the rest:
                # the first matmuls only wait on ~2.2MB of DMA.
                xch = [xc_dma(0)]
                nc.sync.dma_start(out=wk_sb[:], in_=wkT_d[:])
                nc.sync.dma_start(out=wv_sb[:], in_=wvT_d[:])
                nc.sync.dma_start(out=wq_sb[:, :, 0:P], in_=wqT_d[:, :, 0:P])
                xch += [xc_dma(1), xc_dma(2), xc_dma(3)]

                for nb in range(NB):
                    sl = slice(nb * 512, (nb + 1) * 512)
                    qps = pp.tile([P, 512], f32, tag="qp0", bufs=2)
                    kvps = pkv.tile([P, 1024], f32, tag="kvp", bufs=2)
                    kps = kvps[:, 0:512]
                    vps = kvps[:, 512:1024]
                    for e in range(ET):
                        if e == 8 and nb < 2:  # prefetch one full chunk ahead
                            xch.append(xc_dma(2 * nb + 4))
                            xch.append(xc_dma(2 * nb + 5))
                        xc = xch[2 * nb + e // 8]
                        xce = xc[:, e % 8, :]
                        st = e == 0
                        sp = e == ET - 1
                        nc.tensor.matmul(
                            kps, wk_sb[:, e, :], xce, start=st, stop=sp
                        )
                        nc.tensor.matmul(
                            vps, wv_sb[:, e, :], xce, start=st, stop=sp
                        )
                        nc.tensor.matmul(
                            qps[:],
                            wq_sb[:, e, 0:P],
                            xce,
                            start=st,
                            stop=sp,
                        )
                    nc.vector.tensor_copy(qT[0][:, sl], qps[:])
                    # k/v copies on ACT (idle in phase 1) so DVE stays free
                    nc.scalar.activation(
                        kT[:, sl], kps, mybir.ActivationFunctionType.Copy
                    )
                    nc.scalar.activation(
                        vTs[:, sl], vps, mybir.ActivationFunctionType.Copy
                    )
                    # v transpose for this chunk's 4 key tiles, via a plain
                    # matmul against identity (background weight-load) rather
                    # than is_transpose (whose LDWEIGHTS can't hide)
                    # v transpose for this chunk's 4 key tiles
                    for j in range(nb * 4, nb * 4 + 4):
                        tp = ptr0.tile([P, P], bf16, tag="tp0", bufs=2)
                        nc.tensor.transpose(
                            tp[:], vTs[:, j * P:(j + 1) * P], ident[:]
                        )
                        nc.vector.tensor_copy(va[:, j, 0:P], tp[:])
                nc.vector.memset(va[:, :, P:P + 1], 1.0)

            # ---------------- phase 2 + 3: attention with interleaved
            # output projection ----------------
            blocks = [(ib, g) for ib in range(IB2) for g in range(G)]

            with tc.tile_pool(name="et", bufs=2) as etp, \
                 tc.tile_pool(name="small", bufs=4) as small, \
                 tc.tile_pool(name="op", bufs=4) as op, \
                 tc.tile_pool(name="ps", bufs=2, space="PSUM") as ps, \
                 tc.tile_pool(name="pav", bufs=2, space="PSUM") as pav, \
                 tc.tile_pool(name="ptr", bufs=1, space="PSUM") as ptr, \
                 tc.tile_pool(name="po", bufs=1, space="PSUM") as po:

                ets_of = {}  # bi -> list of 16 ets tiles

                def scores_unit(bi, j):
                    ib, g = blocks[bi]
                    sps = ps.tile([P, 1024], f32, tag="sps")
                    for half in range(2):
                        nc.tensor.matmul(
                            sps[:, half * 512:(half + 1) * 512],
                            kT[:, j * P:(j + 1) * P],
                            qT[g][:, ib * 1024 + half * 512:
                                   ib * 1024 + (half + 1) * 512],
                            start=True,
                            stop=True,
                        )
                    et = etp.tile([P, 1024], bf16, name=f"et{j}", tag=f"et{j}")
                    nc.scalar.activation(
                        et[:],
                        sps[:],
                        mybir.ActivationFunctionType.Exp,
                        scale=SCALE,
                    )
                    ets_of[bi][j] = et

                avp_of = {}  # (bi, isub) -> psum tile

                def av_unit(bi, u):
                    # u in 0..15: two units per isub; unit 2k = j 0..7 of
                    # isub k, unit 2k+1 = j 8..15 + the normalize chain.
                    ib, g = blocks[bi]
                    isub = u // 2
                    ets = ets_of[bi]
                    if u % 2 == 0:
                        avp = pav.tile([P, 132], f32, tag="avp")
                        avp_of[(bi, isub)] = avp
                        jr = range(0, 8)
                    else:
                        avp = avp_of[(bi, isub)]
                        jr = range(8, 16)
                    for j in jr:
                        nc.tensor.matmul(
                            avp[:, 0:129],
                            ets[j][:, isub * P:(isub + 1) * P],
                            va[:, j, 0:129],
                            start=(j == 0),
                            stop=(j == JT - 1),
                        )
                    if u % 2 == 1:
                        rec = small.tile([P, 1], f32, tag="rec")
                        nc.vector.reciprocal(rec[:], avp[:, 128:129])
                        anorm = small.tile([P, P], bf16, tag="anorm")
                        nc.vector.tensor_scalar_mul(anorm[:], avp[:, 0:P], rec[:])
                        trp = ptr.tile([P, P], bf16, tag="trp")
                        nc.tensor.transpose(trp[:], anorm[:], ident[:])
                        col = (ib * 8 + isub) * P
                        nc.vector.tensor_copy(attnT[g][:, col:col + P], trp[:])

                def ph3_sub(eo, nb, pool, ot_pool, ot_tag, pot_tag, pot_bufs):
                    # one PSUM accumulation group + copy + store
                    pot = pool.tile([P, 512], f32, tag=pot_tag, bufs=pot_bufs)
                    for f in range(G):
                        nc.tensor.matmul(
                            pot[:],
                            wo_sb[:, f, eo * P:(eo + 1) * P],
                            attnT[f][:, nb * 512:(nb + 1) * 512],
                            start=(f == 0),
                            stop=(f == G - 1),
                        )
                    ot = ot_pool.tile([P, 512], bf16, tag=ot_tag)
                    nc.vector.tensor_copy(ot[:], pot[:])
                    nc.sync.dma_start(
                        out=outT_d[eo, :, nb * 512:(nb + 1) * 512],
                        in_=ot[:],
                    )

                # Deferred Q projections for g1..g3, hosted as PE filler in
                # blocks 0..2: block bi computes Q[bi+1] in 4 chunks of 4
                # j-steps (4 e-slices per step), x chunks re-DMA'd through
                # the same ring with 2-chunk prefetch.
                qsp = {}

                # Q[g] halves 0-3 (columns 0:1024) are needed by block g;
                # halves 4-7 by block 4+g.  Spreading per this schedule keeps
                # every pre-ph3 block supplied with PE filler.
                QSCHED = {
                    0: [(1, 0), (1, 1), (1, 2), (1, 3), (2, 0), (2, 1)],
                    1: [(2, 2), (2, 3), (3, 0), (3, 1)],
                    2: [(3, 2), (3, 3), (1, 4), (1, 5)],
                    3: [(1, 6), (1, 7), (2, 4), (2, 5)],
                    4: [(2, 6), (2, 7), (3, 4), (3, 5), (3, 6), (3, 7)],
                }
                XORDER = [h for b in range(5) for (g, h) in QSCHED[b]]
                qstate = {"idx": 0}

                def q_sub(gh, sub, xq):
                    # one quarter of a Q[g] column-chunk: 4 contraction MMs
                    g, h = gh
                    if sub == 0:
                        qsp["cur"] = xq.pop(0)
                        nxt = qstate["idx"] + 4
                        if nxt < len(XORDER):
                            xq.append(xc_dma(XORDER[nxt], split=False))
                        qstate["idx"] += 1
                        if h % 2 == 0:
                            qsp["pot"] = po.tile([P, 512], f32, name="qpot",
                                                 tag="pot", bufs=1)
                    xc, pot = qsp["cur"], qsp["pot"]
                    es = (h % 2) * 8 + sub * 4
                    for e in range(es, es + 4):
                        nc.tensor.matmul(
                            pot[:],
                            wq_sb[:, e, g * P:(g + 1) * P],
                            xc[:, e % 8, :],
                            start=(e == 0),
                            stop=(e == ET - 1),
                        )
                    if es == 12:
                        sc = h // 2
                        nc.vector.tensor_copy(
                            qT[g][:, sc * 512:(sc + 1) * 512], pot[:]
                        )

                # ph3 for half 0 can only be emitted once block 3's AV units
                # are all emitted (in-order PE: a waiting filler would block
                # the stream), i.e. from block 5 on. One PSUM group per
                # filler slot so the single po bank recycles between slots.
                half0 = [(eo, nb) for eo in range(ET) for nb in range(2)]

                xq = [xc_dma(h, split=False) for h in XORDER[0:2]]
                # deferred q-head weights for g1/g2 land next, then the rest
                # of the filler x halves, g3 weights, and wo
                nc.sync.dma_start(out=wq_sb[:, :, P:3 * P],
                                  in_=wqT_d[:, :, P:3 * P])
                xq += [xc_dma(h, split=False) for h in XORDER[2:4]]
                nc.sync.dma_start(out=wq_sb[:, :, 3 * P:FQ],
                                  in_=wqT_d[:, :, 3 * P:FQ])
                nc.sync.dma_start(out=wo_sb[:], in_=woT_d[:])
                for bi in range(len(blocks)):
                    ets_of[bi] = [None] * JT
                    qs = QSCHED.get(bi, [])
                    nsub = 2 * len(qs)
                    qslots = {}
                    for k in range(nsub):
                        qslots.setdefault(k * JT // nsub, []).append(
                            (qs[k // 2], k % 2)
                        )
                    for j in range(JT):
                        scores_unit(bi, j)
                        if bi > 0:
                            av_unit(bi - 1, j)
                        for gh, sub in qslots.get(j, []):
                            q_sub(gh, sub, xq)
                        if bi >= 5 and j % 4 == 3 and half0:
                            eo, nb = half0.pop(0)
                            ph3_sub(eo, nb, po, op, "ot", "pot", 1)
                    if bi >= 2:
                        ets_of.pop(bi - 2)
                # last block's AV + some leftover half-0 ph3
                for u in range(JT):
                    av_unit(len(blocks) - 1, u)
                    if u % 2 == 1 and half0:
                        eo, nb = half0.pop(0)
                        ph3_sub(eo, nb, po, op, "ot", "pot", 1)

                tail = half0 + [(eo, 2 + nbh)
                                for eo in range(ET) for nbh in range(2)]

            # ---- phase 3 tail: remaining half-0 + all half-1 subunits,
            # multi-banked PSUM for dense back-to-back groups
            with tc.tile_pool(name="po2", bufs=3, space="PSUM") as po2, \
                 tc.tile_pool(name="op2", bufs=4) as op2:
                for eo, nb in tail:
                    pot = po2.tile([P, 512], f32, tag="pot2")
                    for f in range(G):
                        nc.tensor.matmul(
                            pot[:],
                            wo_sb[:, f, eo * P:(eo + 1) * P],
                            attnT[f][:, nb * 512:(nb + 1) * 512],
                            start=(f == 0),
                            stop=(f == G - 1),
                        )
                    ot = op2.tile([P, 512], bf16, tag="ot2")
                    nc.vector.tensor_copy(ot[:], pot[:])
                    nc.sync.dma_start(
                        out=outT_d[eo, :, nb * 512:(nb + 1) * 512],
                        in_=ot[:],
                    )
    nc.finalize()
    return nc


def _get_program():
    if "nc" not in _CACHE:
        _CACHE["nc"] = _build_program()
    return _CACHE["nc"]


def _make_in_maps(x, Wq, Wk, Wv, Wo):
    import ml_dtypes

    bf = ml_dtypes.bfloat16

    def wtile(w):  # [rows, E] -> [P, ET_rows, rows_per] tiled on partition
        r = w.shape[0]
        return np.ascontiguousarray(
            w.T.reshape(ET, P, r).transpose(1, 0, 2)
        ).astype(bf)

    # x chunks: [NB, P, ET, 512]; element (nb,p,e,c) = x^T[e*128+p, nb*512+c]
    xC = [
        np.ascontiguousarray(
            x[b].T.reshape(ET, P, NB, 512).transpose(2, 1, 0, 3)
        ).astype(bf)
        for b in range(B)
    ]
    in_maps = []
    for c in range(8):
        b, h = c // HKV, c % HKV
        wo = Wo[:, h * FQ:(h + 1) * FQ].T  # [FQ, E]
        in_maps.append({
            "xC": xC[b],
            "wqT": wtile(Wq[h * FQ:(h + 1) * FQ, :]),
            "wkT": wtile(Wk[h * D:(h + 1) * D, :]),
            "wvT": wtile(Wv[h * D:(h + 1) * D, :]),
            "woT": np.ascontiguousarray(
                wo.reshape(G, P, N).transpose(1, 0, 2)
            ).astype(bf),
        })
    return in_maps


def run_spmd(in_maps, trace=False, **kw):
    from concourse.bass_utils import run_bass_kernel_spmd

    nc = _get_program()
    return run_bass_kernel_spmd(nc, in_maps, list(range(8)), trace=trace, **kw)


def kernel(x, Wq, Wk, Wv, Wo, next_token_only=0, **_ignored):
    x = np.asarray(x, dtype=np.float32)
    Wq = np.asarray(Wq, dtype=np.float32)
    Wk = np.asarray(Wk, dtype=np.float32)
    Wv = np.asarray(Wv, dtype=np.float32)
    Wo = np.asarray(Wo, dtype=np.float32)

    res = run_spmd(_make_in_maps(x, Wq, Wk, Wv, Wo))
    outs = [np.asarray(r["outT"]).reshape(E, N).astype(np.float32)
            for r in res.results]
    full = np.empty((B, N, E), np.float32)
    for b in range(B):
        acc = outs[b * HKV]
        for h in range(1, HKV):
            acc = acc + outs[b * HKV + h]
        full[b] = acc.T
    return full
